# revision 8
# baseline (speedup 1.0000x reference)
"""Trainium2 Bass kernel for a 3-layer bidirectional LSTM + dense head model.

Takes full (unsharded) inputs, shards the batch across 8 NeuronCores
(pure data parallel), runs a hand-written Bass/Tile kernel, gathers the
full output.

Layout strategy (per core, batch 1024 = 2 chunks x 512 columns):
  - feature-major everywhere: features on SBUF partitions, batch in free dim
  - LSTM step: z = Wcat.T @ [x_t; h] accumulated in PSUM, gates via
    ScalarE sigmoid / relu, c/h updates on VectorE, all bf16 storage
  - L1 (u=64): fwd dir lives on partitions 0:64, bwd on 64:128; recurrent
    weights are block-diagonal so one matmul serves both directions
  - dense d1 (15360->256) is fused into the L3 scan: each step's h3 is
    immediately matmul-accumulated against the matching d1_w row slice
  - BatchNorm (inference) folded into the final 64->26 layer on host
"""

import os
import sys

for _p in ("/opt/trn_rl_repo", "/root/.axon_site/_ro/trn_rl_repo"):
    if os.path.isdir(_p) and _p not in sys.path:
        sys.path.insert(0, _p)

import ml_dtypes
import numpy as np

import concourse.bass as bass
import concourse.bacc as bacc
import concourse.mybir as mybir
import concourse.tile as tile
from concourse.bass import ds, ts
from concourse.bass_utils import run_bass_kernel_spmd

BF = mybir.dt.bfloat16
F8 = mybir.dt.float8e4
F32 = mybir.dt.float32
AF = mybir.ActivationFunctionType
ALU = mybir.AluOpType
NPBF = ml_dtypes.bfloat16
NPF8 = ml_dtypes.float8_e4m3
DR = mybir.MatmulPerfMode.DoubleRow

NCORES = 8
BN_EPS = 1e-3

# gate chunk order on chip: (i, g, f, o); reference kernels use (i, f, g, o)
GATE_PERM = (0, 2, 1, 3)


class Cfg:
    def __init__(self, B=8192, T=30, D=126, C=512, NCH=2,
                 U=(64, 128, 256), NC1=256, NC2=128, NC3=64, NCLS=26):
        self.B, self.T, self.D, self.C, self.NCH = B, T, D, C, NCH
        self.U = U
        self.NC1, self.NC2, self.NC3, self.NCLS = NC1, NC2, NC3, NCLS
        self.BC = B // NCORES          # rows per core
        assert self.BC == C * NCH
        self.RG = C // 128             # row-groups of 128 per chunk


DEFAULT_CFG = Cfg()


def _gate_cols(u):
    """Column indices that reorder 4u gate columns from (i,f,g,o) to (i,f,o,g)."""
    idx = []
    for g in GATE_PERM:
        idx.extend(range(g * u, (g + 1) * u))
    return np.array(idx)


def build_nc(cfg: Cfg) -> bass.Bass:
    T, D, C, NCH = cfg.T, cfg.D, cfg.C, cfg.NCH
    u1 = cfg.U[0]
    u3 = cfg.U[2]
    K3 = u3 // 128                     # k-chunks for layer-3 features (2)
    M3 = 4 * u3 // 128                 # m-chunks for layer-3 gates (8)
    NC1, NC2, NC3, NCLS = cfg.NC1, cfg.NC2, cfg.NC3, cfg.NCLS
    MD1 = NC1 // 128                   # d1 output chunks (2)
    RG = cfg.RG

    nc = bacc.Bacc("TRN2", target_bir_lowering=False, debug=False)

    # ---- DRAM I/O ----------------------------------------------------
    xt = nc.dram_tensor("xt", [D, NCH, T, C], BF, kind="ExternalInput")
    y = nc.dram_tensor("y", [NCH * C, NCLS], F32, kind="ExternalOutput")

    w1x_d = nc.dram_tensor("w1x", [D, 4, 128], BF, kind="ExternalInput")
    u1_d = nc.dram_tensor("u1", [128, 4, 128], BF, kind="ExternalInput")
    b1_d = nc.dram_tensor("b1", [128, 4], F32, kind="ExternalInput")

    w2_d = [nc.dram_tensor(f"w2{s}", [128, 4, 128], BF, kind="ExternalInput") for s in "fb"]
    u2_d = [nc.dram_tensor(f"u2{s}", [128, 4, 128], BF, kind="ExternalInput") for s in "fb"]
    b2_d = [nc.dram_tensor(f"b2{s}", [128, 4], F32, kind="ExternalInput") for s in "fb"]

    w3_d = [nc.dram_tensor(f"w3{s}", [128, K3, M3, 128], F8, kind="ExternalInput") for s in "fb"]
    u3_d = [nc.dram_tensor(f"u3{s}", [128, K3, M3, 128], F8, kind="ExternalInput") for s in "fb"]
    b3_d = [nc.dram_tensor(f"b3{s}", [128, M3], F32, kind="ExternalInput") for s in "fb"]

    d1w_d = nc.dram_tensor("d1w", [T, 2, 128, K3, MD1, 128], F8, kind="ExternalInput")
    bd1_d = nc.dram_tensor("bd1", [128, MD1], F32, kind="ExternalInput")
    d2w_d = nc.dram_tensor("d2w", [128, MD1, NC2], BF, kind="ExternalInput")
    bd2_d = nc.dram_tensor("bd2", [NC2, 1], F32, kind="ExternalInput")
    d3w_d = nc.dram_tensor("d3w", [NC2, NC3], BF, kind="ExternalInput")
    bd3_d = nc.dram_tensor("bd3", [NC3, 1], F32, kind="ExternalInput")
    oww_d = nc.dram_tensor("oww", [NC3 + 1, NCLS], BF, kind="ExternalInput")

    with tile.TileContext(nc) as tc:
        with (
            tc.tile_pool(name="const", bufs=1) as const,
            tc.tile_pool(name="xin", bufs=8) as xin,
            tc.tile_pool(name="h1p", bufs=1) as h1p,
            tc.tile_pool(name="h2p", bufs=1) as h2p,
            tc.tile_pool(name="st", bufs=1) as st,
            tc.tile_pool(name="zsb", bufs=1) as zsb,
            tc.tile_pool(name="d1wp", bufs=6) as d1wp,
            tc.tile_pool(name="hd", bufs=2) as hd,
            tc.tile_pool(name="smx", bufs=2) as smx,
            tc.tile_pool(name="zp", bufs=6, space="PSUM") as zp,
            tc.tile_pool(name="d1p", bufs=1, space="PSUM") as d1p,
        ):
            # ---- static weights into SBUF -----------------------------
            def load(dram, dt, nm):
                t = const.tile(list(dram.shape), dt, tag=nm, name=nm)
                nc.sync.dma_start(out=t[:], in_=dram[:])
                return t

            w1x = load(w1x_d, BF, "w1x_sb")
            u1t = load(u1_d, BF, "u1_sb")
            b1 = load(b1_d, F32, "b1_sb")
            w2 = [load(w2_d[d], BF, f"w2_sb{d}") for d in range(2)]
            u2 = [load(u2_d[d], BF, f"u2_sb{d}") for d in range(2)]
            b2 = [load(b2_d[d], F32, f"b2_sb{d}") for d in range(2)]
            # layer-3 weights: fp8, [128, Ko=K3, M3, 128] for DoubleRow
            w3 = [load(w3_d[d], F8, f"w3_sb{d}") for d in range(2)]
            u3t = [load(u3_d[d], F8, f"u3_sb{d}") for d in range(2)]
            b3 = [load(b3_d[d], F32, f"b3_sb{d}") for d in range(2)]
            bd1 = load(bd1_d, F32, "bd1_sb")
            d2w = load(d2w_d, BF, "d2w_sb")
            bd2 = load(bd2_d, F32, "bd2_sb")
            d3w = load(d3w_d, BF, "d3w_sb")
            bd3 = load(bd3_d, F32, "bd3_sb")
            oww = load(oww_d, BF, "oww_sb")

            r3_tiles = []

            for ch in range(NCH):
                # ======== Layer 1 (u=64, both dirs packed on partitions) ====
                h1T = h1p.tile([128, T, C], BF, tag="h1T", name=f"h1T_{ch}")
                h1st = st.tile([128, C], BF, tag="h1st", name=f"h1st_{ch}")
                c1 = st.tile([128, C], BF, tag="c1", name=f"c1_{ch}")
                z1 = [zsb.tile([128, C], BF, tag=f"z1_{g}", name=f"z1_{ch}_{g}")
                      for g in range(4)]

                for s in range(T):
                    tf, tb = s, T - 1 - s
                    xf = xin.tile([D, C], BF, tag="xf", name=f"xf_{ch}_{s}")
                    nc.sync.dma_start(out=xf[:], in_=xt[:, ch, tf])
                    xb = xin.tile([D, C], BF, tag="xb", name=f"xb_{ch}_{s}")
                    nc.sync.dma_start(out=xb[:], in_=xt[:, ch, tb])

                    zts = [zp.tile([128, C], F32, tag="zt", name=f"zt1_{ch}_{s}_{m}")
                           for m in range(4)]
                    for m in range(4):
                        zt = zts[m]
                        last = s == 0
                        nc.tensor.matmul(zt[0:u1, :], w1x[:, m, 0:u1], xf[:],
                                         start=True, stop=last, skip_group_check=True)
                        nc.tensor.matmul(zt[u1:128, :], w1x[:, m, u1:128], xb[:],
                                         start=True, stop=last, skip_group_check=True)
                        if s > 0:
                            nc.tensor.matmul(zt[:], u1t[:, m, :], h1st[:],
                                             start=False, stop=True, skip_group_check=True)
                    # gates
                    nc.scalar.activation(z1[0][:], zts[0][:], AF.Sigmoid, bias=b1[:, 0:1])
                    nc.vector.tensor_scalar(z1[1][:], zts[1][:], b1[:, 1:2], 0.0,
                                            op0=ALU.add, op1=ALU.max)
                    nc.scalar.activation(z1[2][:], zts[2][:], AF.Sigmoid, bias=b1[:, 2:3])
                    nc.scalar.activation(z1[3][:], zts[3][:], AF.Sigmoid, bias=b1[:, 3:4])
                    # c/h update  (0=i 1=g 2=f 3=o)
                    if s == 0:
                        nc.vector.tensor_tensor(c1[:], z1[0][:], z1[1][:], op=ALU.mult)
                    else:
                        nc.vector.tensor_tensor(z1[1][:], z1[0][:], z1[1][:], op=ALU.mult)
                        nc.vector.tensor_tensor(z1[2][:], z1[2][:], c1[:], op=ALU.mult)
                        nc.vector.tensor_tensor(c1[:], z1[2][:], z1[1][:], op=ALU.add)
                    # h = relu(c) * o
                    nc.vector.scalar_tensor_tensor(h1st[:], c1[:], 0.0, z1[3][:],
                                                   op0=ALU.max, op1=ALU.mult)
                    nc.vector.tensor_copy(h1T[0:u1, tf, :], h1st[0:u1, :])
                    nc.vector.tensor_copy(h1T[u1:128, tb, :], h1st[u1:128, :])

                # ======== Layer 2 (u=128, per-direction) ====================
                h2T8 = h2p.tile([128, 2, T, C], F8, tag="h2T8", name=f"h2T8_{ch}")
                h2st = [st.tile([128, C], BF, tag=f"h2st_{d}", name=f"h2st_{ch}_{d}")
                        for d in range(2)]
                c2 = [st.tile([128, C], BF, tag=f"c2_{d}", name=f"c2_{ch}_{d}")
                      for d in range(2)]
                z2 = [[zsb.tile([128, C], BF, tag=f"z2_{d}_{g}", name=f"z2_{ch}_{d}_{g}")
                       for g in range(4)] for d in range(2)]

                for s in range(T):
                    for d in range(2):
                        td = s if d == 0 else T - 1 - s
                        zts = [zp.tile([128, C], F32, tag="zt", name=f"zt2_{ch}_{s}_{d}_{m}")
                               for m in range(4)]
                        for m in range(4):
                            zt = zts[m]
                            nc.tensor.matmul(zt[:], w2[d][:, m, :], h1T[:, td, :],
                                             start=True, stop=(s == 0))
                            if s > 0:
                                nc.tensor.matmul(zt[:], u2[d][:, m, :], h2st[d][:],
                                                 start=False, stop=True)
                        zg = z2[d]
                        nc.scalar.activation(zg[0][:], zts[0][:], AF.Sigmoid, bias=b2[d][:, 0:1])
                        nc.vector.tensor_scalar(zg[1][:], zts[1][:], b2[d][:, 1:2], 0.0,
                                                op0=ALU.add, op1=ALU.max)
                        nc.scalar.activation(zg[2][:], zts[2][:], AF.Sigmoid, bias=b2[d][:, 2:3])
                        nc.scalar.activation(zg[3][:], zts[3][:], AF.Sigmoid, bias=b2[d][:, 3:4])
                        if s == 0:
                            nc.vector.tensor_tensor(c2[d][:], zg[0][:], zg[1][:], op=ALU.mult)
                        else:
                            nc.vector.tensor_tensor(zg[1][:], zg[0][:], zg[1][:], op=ALU.mult)
                            nc.vector.tensor_tensor(zg[2][:], zg[2][:], c2[d][:], op=ALU.mult)
                            nc.vector.tensor_tensor(c2[d][:], zg[2][:], zg[1][:], op=ALU.add)
                        nc.vector.scalar_tensor_tensor(h2st[d][:], c2[d][:], 0.0,
                                                       zg[3][:], op0=ALU.max, op1=ALU.mult)
                        nc.vector.tensor_copy(h2T8[:, d, td, :], h2st[d][:])

                # ======== Layer 3 (u=256, per-direction) + fused d1 =========
                out1 = d1p.tile([128, MD1, C], F32, tag="out1", name=f"out1_{ch}")
                c3 = [st.tile([128, K3, C], BF, tag=f"c3_{d}", name=f"c3_{ch}_{d}")
                      for d in range(2)]
                h3 = [st.tile([128, K3, C], F8, tag=f"h3_{d}", name=f"h3_{ch}_{d}")
                      for d in range(2)]
                z3 = [[zsb.tile([128, K3, C], BF, tag=f"z3_{d}_{g}", name=f"z3_{ch}_{d}_{g}")
                       for g in range(4)] for d in range(2)]

                for s in range(T):
                    for d in range(2):
                        td = s if d == 0 else T - 1 - s
                        d1t = d1wp.tile([128, K3, MD1, 128], F8, tag="d1t",
                                        name=f"d1t_{ch}_{s}_{d}")
                        nc.sync.dma_start(out=d1t[:], in_=d1w_d[td, d])
                        zts = [zp.tile([128, C], F32, tag="zt",
                                       name=f"zt3_{ch}_{s}_{d}_{m}")
                               for m in range(4 * K3)]
                        for g in range(4):
                            for sub in range(K3):
                                m = g * K3 + sub
                                zt = zts[m]
                                nc.tensor.matmul(zt[:], w3[d][:, :, m, :],
                                                 h2T8[:, :, td, :],
                                                 start=True, stop=(s == 0), perf_mode=DR)
                                if s > 0:
                                    nc.tensor.matmul(zt[:], u3t[d][:, :, m, :], h3[d][:],
                                                     start=False, stop=True, perf_mode=DR)
                        zg = z3[d]
                        for g in range(4):
                            for sub in range(K3):
                                m = g * K3 + sub
                                if g == 1:
                                    nc.vector.tensor_scalar(zg[g][:, sub, :], zts[m][:],
                                                            b3[d][:, m:m + 1], 0.0,
                                                            op0=ALU.add, op1=ALU.max)
                                else:
                                    nc.scalar.activation(zg[g][:, sub, :], zts[m][:],
                                                         AF.Sigmoid, bias=b3[d][:, m:m + 1])
                        if s == 0:
                            nc.vector.tensor_tensor(c3[d][:], zg[0][:], zg[1][:], op=ALU.mult)
                        else:
                            nc.vector.tensor_tensor(zg[1][:], zg[0][:], zg[1][:], op=ALU.mult)
                            nc.vector.tensor_tensor(zg[2][:], zg[2][:], c3[d][:], op=ALU.mult)
                            nc.vector.tensor_tensor(c3[d][:], zg[2][:], zg[1][:], op=ALU.add)
                        nc.vector.scalar_tensor_tensor(h3[d][:], c3[d][:], 0.0, zg[3][:],
                                                       op0=ALU.max, op1=ALU.mult)
                        # fused d1 accumulation over (t, dir)
                        for mc in range(MD1):
                            nc.tensor.matmul(
                                out1[:, mc, :], d1t[:, :, mc, :], h3[d][:],
                                start=(s == 0 and d == 0),
                                stop=(s == T - 1 and d == 1),
                                perf_mode=DR, skip_group_check=True)

                # ======== dense head (relu chain), softmax deferred =========
                out1_sb = hd.tile([128, MD1, C], BF, tag="out1_sb", name=f"out1_sb_{ch}")
                for mc in range(MD1):
                    nc.scalar.activation(out1_sb[:, mc, :], out1[:, mc, :], AF.Relu,
                                         bias=bd1[:, mc:mc + 1])
                zd2 = zp.tile([128, C], F32, tag="zt", name=f"zd2_{ch}")
                for kc in range(MD1):
                    nc.tensor.matmul(zd2[0:NC2, :], d2w[:, kc, :], out1_sb[:, kc, :],
                                     start=(kc == 0), stop=(kc == MD1 - 1))
                out2_sb = hd.tile([128, C], BF, tag="out2_sb", name=f"out2_sb_{ch}")
                nc.scalar.activation(out2_sb[0:NC2, :], zd2[0:NC2, :], AF.Relu,
                                     bias=bd2[:, 0:1])
                zd3 = zp.tile([128, C], F32, tag="zt", name=f"zd3_{ch}")
                nc.tensor.matmul(zd3[0:NC3, :], d3w[:, 0:NC3], out2_sb[0:NC2, :],
                                 start=True, stop=True)
                r3 = hd.tile([128, C], BF, tag="r3", name=f"r3_{ch}")
                nc.scalar.activation(r3[0:NC3, :], zd3[0:NC3, :], AF.Relu,
                                     bias=bd3[:, 0:1])
                nc.vector.memset(r3[NC3:NC3 + 1, :], 1.0)
                r3_tiles.append(r3)

            # ======== logits + softmax (fp32) ==============================
            for ch in range(NCH):
                r3 = r3_tiles[ch]
                lg = zp.tile([128, RG, NCLS], F32, tag="zt", name=f"lg_{ch}")
                for r in range(RG):
                    nc.tensor.matmul(lg[:, r, :], r3[0:NC3 + 1, ts(r, 128)],
                                     oww[0:NC3 + 1, :], start=True, stop=True)
                mx = smx.tile([128, RG, 1], F32, tag="mx", name=f"mx_{ch}")
                nc.vector.reduce_max(out=mx[:], in_=lg[:], axis=mybir.AxisListType.X)
                e = smx.tile([128, RG, NCLS], F32, tag="e", name=f"e_{ch}")
                for r in range(RG):
                    nc.vector.tensor_scalar(e[:, r, :], lg[:, r, :], mx[:, r, :], None,
                                            op0=ALU.subtract)
                nc.scalar.activation(e[:], e[:], AF.Exp)
                sm = smx.tile([128, RG, 1], F32, tag="sm", name=f"sm_{ch}")
                nc.vector.reduce_sum(out=sm[:], in_=e[:], axis=mybir.AxisListType.X)
                rs = smx.tile([128, RG, 1], F32, tag="rs", name=f"rs_{ch}")
                nc.vector.reciprocal(rs[:], sm[:])
                yo = smx.tile([128, RG, NCLS], F32, tag="yo", name=f"yo_{ch}")
                for r in range(RG):
                    nc.vector.tensor_scalar(yo[:, r, :], e[:, r, :], rs[:, r, :], None,
                                            op0=ALU.mult)
                nc.sync.dma_start(
                    out=y[ch * C:(ch + 1) * C].rearrange("(r p) n -> p r n", p=128),
                    in_=yo[:])

    if hasattr(nc, 'compile'):
        nc.compile()
    return nc


# ---------------------------------------------------------------------------
# host-side input prep
# ---------------------------------------------------------------------------

def _bf(a):
    return np.ascontiguousarray(np.asarray(a, np.float32).astype(NPBF))


def _f8(a):
    return np.ascontiguousarray(np.asarray(a, np.float32).astype(NPF8))


def _f32(a):
    return np.ascontiguousarray(np.asarray(a, np.float32))


def prep_weights(inp, cfg: Cfg):
    """Build the shared (replicated) weight arrays for the in_maps."""
    T, D = cfg.T, cfg.D
    u1, u2, u3 = cfg.U
    K3, M3 = u3 // 128, 4 * u3 // 128
    NC1, NC2, NC3, NCLS = cfg.NC1, cfg.NC2, cfg.NC3, cfg.NCLS
    MD1 = NC1 // 128
    g1, g2, g3 = _gate_cols(u1), _gate_cols(u2), _gate_cols(u3)

    out = {}
    # L1 combined
    w1f = _f32(inp["w1f"])[:, g1]
    w1b = _f32(inp["w1b"])[:, g1]
    w1x = np.zeros((D, 4, 128), np.float32)
    for m in range(4):
        w1x[:, m, 0:u1] = w1f[:, m * u1:(m + 1) * u1]
        w1x[:, m, u1:128] = w1b[:, m * u1:(m + 1) * u1]
    out["w1x"] = _bf(w1x)
    u1f = _f32(inp["u1f"])[:, g1]
    u1b = _f32(inp["u1b"])[:, g1]
    u1c = np.zeros((128, 4, 128), np.float32)
    for m in range(4):
        u1c[0:u1, m, 0:u1] = u1f[:, m * u1:(m + 1) * u1]
        u1c[u1:128, m, u1:128] = u1b[:, m * u1:(m + 1) * u1]
    out["u1"] = _bf(u1c)
    b1f = _f32(inp["b1f"])[g1]
    b1b = _f32(inp["b1b"])[g1]
    b1 = np.zeros((128, 4), np.float32)
    for m in range(4):
        b1[0:u1, m] = b1f[m * u1:(m + 1) * u1]
        b1[u1:128, m] = b1b[m * u1:(m + 1) * u1]
    out["b1"] = b1

    # L2 per dir
    for sfx in "fb":
        w = _f32(inp[f"w2{sfx}"])[:, g2]
        out[f"w2{sfx}"] = _bf(w.reshape(128, 4, 128))
        uu = _f32(inp[f"u2{sfx}"])[:, g2]
        out[f"u2{sfx}"] = _bf(uu.reshape(128, 4, 128))
        bb = _f32(inp[f"b2{sfx}"])[g2]
        out[f"b2{sfx}"] = np.ascontiguousarray(bb.reshape(4, 128).T)

    # L3 per dir (fp8, partition-first [128, Ko=K3, M3, 128] for DoubleRow)
    for sfx in "fb":
        w = _f32(inp[f"w3{sfx}"])[:, g3]          # [256, 1024]
        out[f"w3{sfx}"] = _f8(w.reshape(K3, 128, M3, 128).transpose(1, 0, 2, 3))
        uu = _f32(inp[f"u3{sfx}"])[:, g3]
        out[f"u3{sfx}"] = _f8(uu.reshape(K3, 128, M3, 128).transpose(1, 0, 2, 3))
        bb = _f32(inp[f"b3{sfx}"])[g3]
        out[f"b3{sfx}"] = np.ascontiguousarray(bb.reshape(M3, 128).T)

    # dense d1: rows indexed by (t, dir, feat)
    d1w = _f32(inp["d1_w"]).reshape(T, 2, K3, 128, MD1, 128)
    out["d1w"] = _f8(d1w.transpose(0, 1, 3, 2, 4, 5))      # [T,dir,128,Ko,MD1,128]
    out["bd1"] = np.ascontiguousarray(_f32(inp["d1_b"]).reshape(MD1, 128).T)
    out["d2w"] = _bf(_f32(inp["d2_w"]).reshape(MD1, 128, NC2).transpose(1, 0, 2))
    out["bd2"] = _f32(inp["d2_b"]).reshape(NC2, 1)
    out["d3w"] = _bf(inp["d3_w"])
    out["bd3"] = _f32(inp["d3_b"]).reshape(NC3, 1)

    # fold BN into output layer
    kg = _f32(inp["bn_g"]) / np.sqrt(_f32(inp["bn_v"]) + BN_EPS)
    ow = _f32(inp["out_w"]) * kg[:, None]
    ob = _f32(inp["out_b"]) + (_f32(inp["bn_b"]) - _f32(inp["bn_m"]) * kg) @ _f32(inp["out_w"])
    oww = np.concatenate([ow, ob[None, :]], axis=0)
    out["oww"] = _bf(oww)
    return out


def prep_x(x_core, cfg: Cfg):
    """[BC, T, D] fp32 -> [D, NCH, T, C] bf16 (feature-major)."""
    a = np.asarray(x_core, np.float32).reshape(cfg.NCH, cfg.C, cfg.T, cfg.D)
    a = a.transpose(3, 0, 2, 1)        # [D, NCH, T, C]
    return _bf(a)


_NC_CACHE = {}


def get_nc(cfg: Cfg):
    key = (cfg.B, cfg.T, cfg.C, cfg.NCH)
    if key not in _NC_CACHE:
        _NC_CACHE[key] = build_nc(cfg)
    return _NC_CACHE[key]


def kernel(**inputs) -> np.ndarray:
    cfg = DEFAULT_CFG
    nc = get_nc(cfg)
    weights = prep_weights(inputs, cfg)
    x = np.asarray(inputs["x"], np.float32)
    in_maps = []
    for c in range(NCORES):
        m = dict(weights)
        m["xt"] = prep_x(x[c * cfg.BC:(c + 1) * cfg.BC], cfg)
        in_maps.append(m)
    res = run_bass_kernel_spmd(nc, in_maps, list(range(NCORES)))
    outs = [r["y"] for r in res.results]
    return np.concatenate(outs, axis=0).astype(np.float32)
